# revision 26
# baseline (speedup 1.0000x reference)
"""Luong attention energies + softmax on 8 TRN2 NeuronCores.

reference math (per core, batch-sharded):
  energy[b,s] = <hid[b], enc[s,b]> + (hid[b] @ A) . emb[s,b]
  out[b,0,s]  = softmax_s(energy[b,s])

Full shapes: hidden [1,64,512] f32, encoder_outputs [2048,64,512] f32,
embedding [2048,64,3] f32, affect_matrix [512,3] f32 -> out [64,1,2048] f32.

Sharding: batch dim 64 -> 8 cores x 8. No cross-core communication.

Per-core plan (memory-bound: 32 MB encoder shard, ~90 us at 358 GB/s):
  GpSimd elementwise is avoided: it shares an SBUF port with the DVE and
  stalls 2-port DVE streams almost 1:1. GpSimd only does broadcasts and
  the emba DMA (SWDGE, off the HWDGE rings the enc stream uses).
  stream enc in 16 per-tile DMAs (2 MB each, bufs=6 lookahead):
    DVE : one grouped mult per tile (all 8 b) -> pd, reduce b0-1
          (+ b2 on even tiles)
    ACT : Copy-with-accum reduces b3-7 (+ b2 on odd tiles), junk out in
          PSUM (ScalarE is closer to PSUM)
  the affect-term chain runs in the DMA shadow of tiles 0-1 so no engine
  queue blocks mid-stream.
  epilogue without the true max: exp(e/2-25) on ACT then squared on DVE
  (= exp(e-50), f32-safe); PE ones-matmul column sums; DVE reciprocal;
  PE transpose puts (b,t) on partitions so the 1/sum is a per-partition
  ACT scale fused into the PSUM->SBUF copy; direct strided store.
"""

import numpy as np

S, B, H, E = 2048, 64, 512, 3
N_CORES = 8
BS = B // N_CORES      # 8 batches per core
NT = S // 128          # 16 s-tiles of 128 rows

_CACHE = {}


def _build_nc():
    import concourse.bass as bass
    import concourse.tile as tile
    from concourse import bacc, mybir
    from concourse.mybir import AluOpType as alu
    from concourse.mybir import ActivationFunctionType as actf

    f32 = mybir.dt.float32

    nc = bacc.Bacc("TRN2", target_bir_lowering=False, debug=False)
    enc = nc.dram_tensor("enc", [S, BS, H], f32, kind="ExternalInput").ap()
    emb = nc.dram_tensor("emb", [S, BS, E], f32, kind="ExternalInput").ap()
    hid = nc.dram_tensor("hid", [1, BS, H], f32, kind="ExternalInput").ap()
    amat = nc.dram_tensor("amat", [H, E], f32, kind="ExternalInput").ap()
    out = nc.dram_tensor("out", [BS, 1, S], f32, kind="ExternalOutput").ap()

    with tile.TileContext(nc) as tc:
        with (
            tc.tile_pool(name="persist", bufs=1) as pp,
            tc.tile_pool(name="enc", bufs=6) as encp,
            tc.tile_pool(name="pd", bufs=4) as pdp,
            tc.tile_pool(name="pjunk", bufs=1, space="PSUM") as pjp,
            tc.tile_pool(name="psum", bufs=2, space="PSUM") as psp,
        ):
            # ---- energy tiles: P col = b*NT + t ----
            EbufD = pp.tile([128, 2 * NT], f32)   # b0..1 (DVE)
            EbufM = pp.tile([128, NT], f32)       # b2 (DVE t%4==0 / ACT else)
            EbufA = pp.tile([128, 5 * NT], f32)   # b3..7 (ACT)
            junkA = pjp.tile([128, H], f32)       # ACT accum main-out (PSUM)

            # ---- prologue: everything with no enc dependency, scheduled
            # before the stream so no engine queue blocks later ----
            hidrow = pp.tile([1, BS * H], f32)
            nc.sync.dma_start(hidrow[:], hid.rearrange("o b h -> o (b h)"))
            hidb = pp.tile([128, BS * H], f32)
            nc.gpsimd.partition_broadcast(hidb[:], hidrow[0:1, :])
            hidb_v = hidb[:].rearrange("p (b h) -> p b h", h=H)
            hid8 = pp.tile([BS, H], f32)
            nc.scalar.dma_start(hid8[:], hid[0])
            # amat in [h', (c, e)] chunks for the PE hA matmuls
            am128 = pp.tile([128, 4 * E], f32)
            nc.gpsimd.dma_start(
                am128[:].rearrange("p (c e) -> p c e", e=E),
                amat.rearrange("(c p) e -> p c e", p=128))
            # identity for the PE transposes + constants
            pidx = pp.tile([128, 1], f32)
            nc.gpsimd.iota(pidx[:], pattern=[[0, 1]], base=0,
                           channel_multiplier=1,
                           allow_small_or_imprecise_dtypes=True)
            colidx = pp.tile([128, 128], f32)
            nc.gpsimd.iota(colidx[:], pattern=[[1, 128]], base=0,
                           channel_multiplier=0,
                           allow_small_or_imprecise_dtypes=True)
            ident = pp.tile([128, 128], f32)
            nc.vector.tensor_scalar(ident[:], colidx[:], pidx[:, 0:1],
                                    None, alu.is_equal)
            ones1 = pp.tile([128, 1], f32)
            nc.vector.memset(ones1[:], 1.0)
            ebias = pp.tile([128, 1], f32)
            nc.vector.memset(ebias[:], -25.0)

            # ---- main loop: one DMA + compute per s-tile ----
            for t in range(NT):
                et = encp.tile([128, BS * H], f32, tag="et")
                et_v = et[:].rearrange("p (b h) -> p b h", h=H)
                nc.sync.dma_start(et_v, enc[t * 128:(t + 1) * 128])

                # per-tile compute
                pd = pdp.tile([128, BS * H], f32, tag="pd")
                pd_v = pd[:].rearrange("p (b h) -> p b h", h=H)
                nc.vector.tensor_tensor(pd_v, et_v, hidb_v, alu.mult)
                nc.vector.tensor_reduce(
                    EbufD[:].rearrange("p (b t) -> p b t", t=NT)[:, :, t:t + 1],
                    pd_v[:, 0:2, :],
                    axis=mybir.AxisListType.X, op=alu.add)
                if t % 2 == 0:   # b2 reduce: every 2nd tile on DVE, rest ACT
                    nc.vector.tensor_reduce(
                        EbufM[:, t:t + 1], pd_v[:, 2, :],
                        axis=mybir.AxisListType.X, op=alu.add)
                else:
                    nc.scalar.activation(
                        junkA[:], pd_v[:, 2, :], actf.Copy,
                        accum_out=EbufM[:, t:t + 1])
                for b in range(3, BS):
                    nc.scalar.activation(
                        junkA[:], pd_v[:, b, :], actf.Copy,
                        accum_out=EbufA[:, (b - 3) * NT + t:(b - 3) * NT + t + 1])

                if t == 1:
                    # ---- affect-term chain in the tile-2.. DMA shadow ----
                    emba = pp.tile([128, NT * BS * E], f32)
                    emba_v = emba[:].rearrange("p (t b e) -> p t b e", b=BS, e=E)
                    nc.gpsimd.dma_start(emba_v, emb.rearrange("(t p) b e -> p t b e", p=128))
                    # hA[b,e] = sum_h hid[b,h] * A[h,e] on the PE:
                    # transpose hid8 into [h', (c, b)] chunks, then 4
                    # accumulating [128,8]x[128,3] matmuls
                    hT_ps = psp.tile([128, 4 * BS], f32, tag="ps")
                    for c in range(4):
                        nc.tensor.transpose(
                            hT_ps[:, c * BS:(c + 1) * BS],
                            hid8[:, c * 128:(c + 1) * 128], ident[0:BS, 0:BS])
                    hT = pp.tile([128, 4 * BS], f32)
                    nc.vector.tensor_copy(hT[:], hT_ps[:])
                    hA_ps = psp.tile([BS, E], f32, tag="ps")
                    for c in range(4):
                        nc.tensor.matmul(
                            hA_ps[:], hT[:, c * BS:(c + 1) * BS],
                            am128[:].rearrange("p (c e) -> p c e", e=E)[:, c, :],
                            start=(c == 0), stop=(c == 3))
                    hA = pp.tile([BS, E], f32)
                    nc.vector.tensor_copy(hA[:], hA_ps[:])
                    harow = pp.tile([1, BS * E], f32)
                    nc.scalar.dma_start(harow[0:1].rearrange("o (b e) -> o b e", e=E), hA[:])
                    hab = pp.tile([128, BS * E], f32)
                    nc.gpsimd.partition_broadcast(hab[:], harow[0:1, :])

                    # aff[p, t, b] = sum_e emb[t*128+p, b, e] * hA[b, e]
                    afftmp = pp.tile([128, NT * BS * E], f32)
                    nc.vector.tensor_tensor(
                        afftmp[:].rearrange("p (t b e) -> p t b e", b=BS, e=E),
                        emba_v,
                        hab[:].rearrange("p (b e) -> p b e", e=E)
                        .unsqueeze(1).broadcast_to([128, NT, BS, E]),
                        alu.mult)
                    aff = pp.tile([128, NT * BS], f32)
                    aff_v = aff[:].rearrange("p (t b) -> p t b", b=BS)
                    nc.vector.tensor_reduce(
                        aff_v, afftmp[:].rearrange("p (t b e) -> p t b e", b=BS, e=E),
                        axis=mybir.AxisListType.X, op=alu.add)

            # ---- epilogue ----
            EbufD_v = EbufD[:].rearrange("p (b t) -> p b t", t=NT)
            EbufA_v = EbufA[:].rearrange("p (b t) -> p b t", t=NT)
            nc.vector.tensor_tensor(
                EbufD_v, EbufD_v, aff_v[:, :, 0:2].transpose([0, 2, 1]), alu.add)
            nc.vector.tensor_tensor(
                EbufM[:].unsqueeze(1), EbufM[:].unsqueeze(1),
                aff_v[:, :, 2:3].transpose([0, 2, 1]), alu.add)
            nc.vector.tensor_tensor(
                EbufA_v, EbufA_v, aff_v[:, :, 3:BS].transpose([0, 2, 1]), alu.add)

            # exp(e/2 - 25) then square = exp(e - 50), f32-safe
            P = pp.tile([128, 128], f32)
            nc.scalar.activation(P[:, 0:2 * NT], EbufD[:], actf.Exp,
                                 bias=ebias[:, 0:1], scale=0.5)
            nc.scalar.activation(P[:, 2 * NT:3 * NT], EbufM[:], actf.Exp,
                                 bias=ebias[:, 0:1], scale=0.5)
            nc.scalar.activation(P[:, 3 * NT:128], EbufA[:], actf.Exp,
                                 bias=ebias[:, 0:1], scale=0.5)
            P2 = pp.tile([128, 128], f32)
            nc.scalar.activation(P2[:], P[:], actf.Square)
            P = P2

            # column sums over the 128 s-partitions: cs[0, b*16+t]
            cs = psp.tile([128, 128], f32, tag="ps")
            nc.tensor.matmul(cs[0:1, :], ones1[:], P[:])
            s8 = pp.tile([1, BS], f32)
            nc.vector.tensor_reduce(
                s8[0:1].rearrange("o b -> o b ()"),
                cs[0:1, :].rearrange("o (b t) -> o b t", t=NT),
                axis=mybir.AxisListType.X, op=alu.add)
            r8 = pp.tile([1, BS], f32)
            nc.vector.reciprocal(r8[:], s8[:])
            rbt = pp.tile([1, 128], f32)
            nc.vector.tensor_copy(
                rbt[0:1].rearrange("o (b t) -> o b t", t=NT),
                r8[0:1].rearrange("o b -> o b ()").broadcast_to([1, BS, NT]))
            # K=1 matmul: rcol[(b,t), 0] = rbt[(b,t)]
            rcol = psp.tile([128, 1], f32, tag="ps")
            nc.tensor.matmul(rcol[:], rbt[:], ones1[0:1, :])
            rcs = pp.tile([128, 1], f32)
            nc.vector.tensor_copy(rcs[:], rcol[:])

            # transpose P to [(b,t), p]; apply 1/sum as a per-partition ACT
            # scale on the PSUM->SBUF copy; store directly
            PT = psp.tile([128, 128], f32, tag="ps")
            nc.tensor.transpose(PT[:], P[:], ident[:])
            osb = pp.tile([128, 128], f32)
            nc.scalar.activation(osb[:], PT[:], actf.Copy, scale=rcs[:, 0:1])
            nc.sync.dma_start(
                out.rearrange("b o (t p) -> (b o t) p", p=128), osb[:])

    nc.compile()
    return nc


def _get_nc():
    if "nc" not in _CACHE:
        _CACHE["nc"] = _build_nc()
    return _CACHE["nc"]


def kernel(hidden, encoder_outputs, embedding, affect_matrix):
    from concourse.bass_utils import run_bass_kernel_spmd

    nc = _get_nc()
    hidden = np.asarray(hidden, dtype=np.float32)
    encoder_outputs = np.asarray(encoder_outputs, dtype=np.float32)
    embedding = np.asarray(embedding, dtype=np.float32)
    affect_matrix = np.asarray(affect_matrix, dtype=np.float32)

    in_maps = []
    for c in range(N_CORES):
        sl = slice(c * BS, (c + 1) * BS)
        in_maps.append({
            "enc": np.ascontiguousarray(encoder_outputs[:, sl, :]),
            "emb": np.ascontiguousarray(embedding[:, sl, :]),
            "hid": np.ascontiguousarray(hidden[:, sl, :]),
            "amat": affect_matrix,
        })
    res = run_bass_kernel_spmd(nc, in_maps, list(range(N_CORES)))
    return np.concatenate([res.results[c]["out"] for c in range(N_CORES)], axis=0)
